# revision 22
# baseline (speedup 1.0000x reference)
"""GCN + DiffPool kernel for Trainium2, data-parallel over graphs across 8 NeuronCores.

Model (per graph, n=150 nodes):
  Z1 = relu(An @ (x @ W1) + b1)          An = D^-1/2 (A+I) D^-1/2
  Z2 = relu(An @ (Z1 @ W2) + b2)
  S  = softmax(An @ (Z2 @ Wa) + ba)      [n, 25]
  Zp = S^T @ Z2 ; Ap = S^T @ (A @ S)
  H  = relu(Anp @ (Zp @ Wp) + bp)        pooled GCN, 25 cluster-nodes
  logits = (sum_rows H) @ Wc + bc

Sharding: 64 graphs -> 8 devices x 8 graphs. The batch adjacency is block
diagonal, so each device only receives its 8 graphs' 150x150 diagonal blocks
(packed into a [128,8,150] + [22,8,150] partition-chunk layout) and its node
rows of x (shipped feature-major). Everything is graph-local; the final [8,10]
logits per device are concatenated on host.

On-device layout convention:
  fm (feature-major): [feat_part, graph, node]  - used for W-multiplies (lhsT)
  nm (node-major):    [node_part, graph, feat]  - used for A-multiplies
A-multiplies contract over nodes, so node dim (150) is split into partition
chunks c0=[0:128], c1=[128:150]. The FULL symmetric normalization is folded
into An = (A+I) * d_i * d_j via a PE rank-1 outer product d d^T per graph;
activations stay unscaled so their PSUM->SBUF evacuations are plain copies
with no dependency on the degree chain.
"""

import numpy as np

import concourse.bass as bass
import concourse.mybir as mybir
import concourse.tile as tile
from concourse import bacc
from concourse.bass_utils import run_bass_kernel_spmd

F32 = mybir.dt.float32
BF16 = mybir.dt.bfloat16
AF = mybir.ActivationFunctionType
AL = mybir.AluOpType
U32 = mybir.dt.uint32

MMDT = BF16

N_NODES = 9600
N_FEAT = 128
HIDDEN = 64
CLUSTERS = 25
NUM_CLASSES = 10
B_GRAPHS = 64
NPG = 150            # nodes per graph
DEV = 8              # devices
GPD = 8              # graphs per device
C0, C1 = 128, 22     # node partition chunks (128 + 22 = 150)
NPGP = 152           # node free-dim padded to 4B alignment for DVE 2x mode
VZ = CLUSTERS + HIDDEN   # fused [v | z2^T] free width = 89

_CACHE = {}

# packed-constant column offsets (bf16 tensor wpk [128, WP_COLS])
WP_W1 = 0                     # [128, 64]
WP_W2 = WP_W1 + HIDDEN        # [64, 64]   at 64
WP_WAID = WP_W2 + HIDDEN      # [64, 25+64] = [Wa | I64] at 128
WP_WP = WP_WAID + VZ          # [64, 64]   at 217
WP_BA = WP_WP + HIDDEN        # [1, 25]    at 281
WP_ID128 = WP_BA + CLUSTERS   # [128, 128] at 306
WP_ONES = WP_ID128 + N_FEAT   # [1, 150]   at 434
WP_ONEC = WP_ONES + NPG       # [128, 1]   at 584
WP_IDP = WP_ONEC + 1          # [128, 2*152] identity chunks at 585
WP_COLS = WP_IDP + 2 * NPGP   # 889

# fp32 tensor fpk [128, FP_COLS]
FP_WC = 0                     # [64, 10]
FP_B1 = FP_WC + NUM_CLASSES   # [64, 1] at 10
FP_B2 = FP_B1 + 1
FP_BP = FP_B2 + 1
FP_BC = FP_BP + 1             # [8, 10] at 13
FP_ID25 = FP_BC + NUM_CLASSES # [25, 25] at 23
FP_ONESR = FP_ID25 + CLUSTERS # [1, 25] ones row at 48
FP_COLS = FP_ONESR + CLUSTERS # 73


def build_nc():
    nc = bacc.Bacc("TRN2", target_bir_lowering=False, debug=False, num_devices=DEV)

    def din(name, shape, dt=F32):
        return nc.dram_tensor(name, shape, dt, kind="ExternalInput").ap()

    xT = din("xT", [N_FEAT, GPD, NPGP], MMDT)
    a0 = din("a0", [C0, GPD, NPGP], MMDT)
    a1 = din("a1", [C1, GPD, NPGP], MMDT)
    wpk = din("wpk", [N_FEAT, WP_COLS], MMDT)
    fpk = din("fpk", [N_FEAT, FP_COLS], F32)
    out = nc.dram_tensor("out", [GPD, NUM_CLASSES], F32, kind="ExternalOutput").ap()

    with tile.TileContext(nc) as tc:
        with (
            tc.tile_pool(name="cst", bufs=1) as cst,
            tc.tile_pool(name="act", bufs=1) as act,
            tc.tile_pool(name="ps", bufs=6, space="PSUM") as ps,
            tc.tile_pool(name="psz", bufs=2, space="PSUM") as psz,
            tc.tile_pool(name="dram", bufs=1, space="DRAM") as dram,
        ):
            # ---- input DMAs: a0 first (heads the degree->An chain); no DMAs
            # on the scalar queue so the ACT table load overlaps the loads ---
            HG = GPD // 2
            s_a0 = cst.tile([C0, GPD, NPGP], MMDT, tag="a0")
            nc.sync.dma_start(out=s_a0[:, 0:HG, :], in_=a0[:, 0:HG, :])
            nc.scalar.dma_start(out=s_a0[:, HG:GPD, :], in_=a0[:, HG:GPD, :])
            s_xT = cst.tile([N_FEAT, GPD, NPGP], MMDT, tag="xT")
            nc.gpsimd.dma_start(out=s_xT[:], in_=xT)
            s_wpk = cst.tile([N_FEAT, WP_COLS], MMDT, tag="wpk")
            nc.gpsimd.dma_start(out=s_wpk[:], in_=wpk)
            s_a1 = cst.tile([C1, GPD, NPGP], MMDT, tag="a1")
            nc.sync.dma_start(out=s_a1[:], in_=a1)
            s_fpk = cst.tile([N_FEAT, FP_COLS], F32, tag="fpk")
            nc.sync.dma_start(out=s_fpk[:], in_=fpk)

            s_a = (s_a0, s_a1)
            s_idp = s_wpk[:, WP_IDP:WP_IDP + 2 * NPGP]
            s_id = (s_idp[:, 0:NPGP], s_idp[0:C1, NPGP:2 * NPGP])
            s_W1 = s_wpk[:, WP_W1:WP_W1 + HIDDEN]
            s_W2 = s_wpk[0:HIDDEN, WP_W2:WP_W2 + HIDDEN]
            s_W2o = s_wpk[HIDDEN:2 * HIDDEN, WP_W2:WP_W2 + HIDDEN]
            s_WAID = s_wpk[0:HIDDEN, WP_WAID:WP_WAID + VZ]
            s_WAIDo = s_wpk[HIDDEN:2 * HIDDEN, WP_WAID:WP_WAID + VZ]
            s_Wp = s_wpk[0:HIDDEN, WP_WP:WP_WP + HIDDEN]
            s_id128 = s_wpk[:, WP_ID128:WP_ID128 + N_FEAT]
            s_onec = s_wpk[:, WP_ONEC:WP_ONEC + 1]
            s_Wc = s_fpk[0:HIDDEN, FP_WC:FP_WC + NUM_CLASSES]
            s_b1d = s_fpk[0:2 * HIDDEN, FP_B1:FP_B1 + 1]
            s_b2d = s_fpk[0:2 * HIDDEN, FP_B2:FP_B2 + 1]
            s_bp = s_fpk[0:HIDDEN, FP_BP:FP_BP + 1]
            s_bc = s_fpk[0:GPD, FP_BC:FP_BC + NUM_CLASSES]
            s_id25 = s_fpk[0:CLUSTERS, FP_ID25:FP_ID25 + CLUSTERS]
            s_ones25 = s_fpk[0:1, FP_ONESR:FP_ONESR + CLUSTERS]

            # ---- degrees (DVE free-dim reduces; pads are zero) -------------
            degc = act.tile([C0, 2 * GPD], F32, tag="degc")
            nc.vector.memset(degc[0:C0, GPD:2 * GPD], 1.0)
            nc.vector.reduce_sum(out=degc[:, 0:HG], in_=s_a0[:, 0:HG, :],
                                 axis=mybir.AxisListType.X)
            nc.vector.reduce_sum(out=degc[:, HG:GPD], in_=s_a0[:, HG:GPD, :],
                                 axis=mybir.AxisListType.X)
            nc.vector.reduce_sum(out=degc[0:C1, GPD:2 * GPD], in_=s_a1[:],
                                 axis=mybir.AxisListType.X)

            # ---- m1 on PE while the rsqrt chain runs on DVE ----------------
            m1 = []
            for c, cn in ((0, C0), (1, C1)):
                off = 0 if c == 0 else C0
                o = act.tile([cn, GPD, HIDDEN], MMDT, tag=f"m1{c}")
                p = ps.tile([cn, GPD, HIDDEN], F32, tag="ps")
                for g in range(GPD):
                    nc.tensor.matmul(p[:, g, :], s_xT[0:N_FEAT, g, off:off + cn],
                                     s_W1, start=True, stop=True)
                nc.scalar.copy(o[:], p[:])
                m1.append(o)

            # ---- PE warm-up filler while the rsqrt chain runs on DVE -------
            p_wu = ps.tile([C0, 3 * NPGP], F32, tag="ps")
            for _ in range(6):
                nc.tensor.matmul(p_wu[:], s_id128, s_xT[:, 0:3, :],
                                 start=True, stop=True)

            # ---- d = rsqrt(deg+1) via ACT: d = exp(-0.5*ln(deg+1)) ---------
            # (two scalar-engine ops per chunk instead of a 10-op serial DVE
            # Newton chain; Ln/Exp/Relu/Copy share one ACT table set)
            lnc = act.tile([C0, 2 * GPD], F32, tag="lnc")
            dbfc = act.tile([C0, 2 * GPD], MMDT, tag="dbfc")
            nc.scalar.activation(lnc[:, 0:GPD], degc[:, 0:GPD], AF.Ln, bias=1.0)
            nc.scalar.activation(dbfc[:, 0:GPD], lnc[:, 0:GPD], AF.Exp,
                                 scale=-0.5)
            nc.scalar.activation(lnc[:, GPD:2 * GPD], degc[:, GPD:2 * GPD],
                                 AF.Ln, bias=1.0)
            nc.scalar.activation(dbfc[:, GPD:2 * GPD], lnc[:, GPD:2 * GPD],
                                 AF.Exp, scale=-0.5)

            # ---- dTrow[g, j] = d_g[j] via PE transposes, then a DRAM
            # roundtrip (split in graph halves, reads on two queues) ---------
            p_dt = ps.tile([GPD, C0], MMDT, tag="ps")
            nc.tensor.transpose(p_dt[:, 0:C0], dbfc[:, 0:GPD], s_id128)
            p_dt2 = ps.tile([GPD, C1], MMDT, tag="ps")
            nc.tensor.transpose(p_dt2[:, 0:C1], dbfc[0:C1, GPD:2 * GPD],
                                s_id128[0:C1, 0:C1])
            dTrow = act.tile([GPD, NPGP], MMDT, tag="dTrow")
            nc.vector.memset(dTrow[:, NPG:NPGP], 0)
            nc.vector.tensor_copy(dTrow[:, 0:C0], p_dt[:])
            nc.vector.tensor_copy(dTrow[:, C0:NPG], p_dt2[:])
            dTd = dram.tile([GPD * NPGP], MMDT, tag="dTd")
            dTd_gj = dTd[:].rearrange("(g j) -> g j", g=GPD)
            nc.sync.dma_start(out=dTd_gj[:], in_=dTrow[:])
            s_dT = cst.tile([C0, GPD, NPGP], MMDT, tag="dT")
            dT_src = dTd_gj[None, :, :].broadcast_to((C0, GPD, NPGP))
            nc.sync.dma_start(out=s_dT[:, 0:HG, :], in_=dT_src[:, 0:HG, :])
            nc.scalar.dma_start(out=s_dT[:, HG:GPD, :], in_=dT_src[:, HG:GPD, :])

            # ---- more warm-up while the dT roundtrip is in flight ----------
            for _ in range(8):
                nc.tensor.matmul(p_wu[:], s_id128, s_xT[:, 0:3, :],
                                 start=True, stop=True)

            # ---- An = (A+I) * d_col * d_row, bf16 (both scalings folded) ---
            ah0 = act.tile([C0, GPD, NPGP], MMDT, tag="ah0")
            idb0 = s_id[0][:, None, :].broadcast_to((C0, GPD, NPGP))
            nc.vector.tensor_add(ah0[:], s_a0[:], idb0)
            ah1 = act.tile([C1, GPD, NPGP], MMDT, tag="ah1")
            idb1 = s_id[1][:, None, :].broadcast_to((C1, GPD, NPGP))
            nc.vector.tensor_add(ah1[:], s_a1[:], idb1)

            # row scaling (broadcast operand, 1x mode) runs during the dT
            # roundtrip; the aligned 2x column scaling lands last.
            an0 = act.tile([C0, GPD, NPGP], MMDT, tag="an0")
            an1 = act.tile([C1 + 1, GPD, NPGP], MMDT, tag="an1")
            dbc0 = dbfc[:, 0:GPD][:, :, None].broadcast_to((C0, GPD, NPGP))
            dbc1 = dbfc[0:C1, GPD:2 * GPD][:, :, None].broadcast_to((C1, GPD, NPGP))
            nc.vector.tensor_mul(ah0[:], ah0[:], dbc0)
            nc.vector.tensor_mul(ah1[:], ah1[:], dbc1)
            for gg in range(2):
                gs = slice(gg * 4, gg * 4 + 4)
                nc.vector.tensor_mul(an0[:, gs, :], ah0[:, gs, :],
                                     s_dT[:, gs, :])
                nc.vector.tensor_mul(an1[0:C1, gs, :], ah1[:, gs, :],
                                     s_dT[0:C1, gs, :])
            s_An = [an0, an1]
            ones_src = wpk[0:1, WP_ONES:WP_ONES + NPG][:, None, :] \
                .broadcast_to((1, GPD, NPG))
            nc.gpsimd.dma_start(out=an1[C1:C1 + 1, :, 0:NPG], in_=ones_src)

            # ---- helpers ---------------------------------------------------
            def an_mult(m_nm, bias, name):
                """fm out [HIDDEN, g, NPG] = relu((An @ M) + bias), batched
                relu over 4-graph PSUM groups."""
                o = act.tile([HIDDEN, GPD, NPG], MMDT, tag=name)
                for gg in range(4):
                    p = psz.tile([HIDDEN, 2, 256], F32, tag="bigz")
                    for g2 in range(2):
                        g = gg * 2 + g2
                        for c, cn in ((0, C0), (1, C1)):
                            nc.tensor.matmul(
                                p[:, g2, 0:NPG], m_nm[c][0:cn, g, :],
                                s_An[c][0:cn, g, 0:NPG],
                                start=(c == 0), stop=(c == 1),
                            )
                    nc.scalar.activation(o[:, gg * 2:gg * 2 + 2, :],
                                         p[:, :, 0:NPG], AF.Relu, bias=bias)
                return o

            def w_mult(lhs_fm, w, kdim, fout, name, extra_row=None, pad=None):
                """nm out [node_chunk, g, fout] = (lhs_fm chunk)^T @ w, plain
                (no scaling).  PSUM->SBUF evacuation on the scalar engine."""
                outs = []
                for c, cn in ((0, C0), (1, C1)):
                    off = 0 if c == 0 else C0
                    rows = cn + (1 if (c == 1 and extra_row is not None) else 0)
                    o = act.tile([rows, GPD, fout], MMDT, tag=f"{name}{c}")
                    if pad is None:
                        p = ps.tile([cn, GPD, fout], F32, tag="ps")
                        for g in range(GPD):
                            nc.tensor.matmul(
                                p[:, g, :], lhs_fm[0:kdim, g, off:off + cn], w,
                                start=True, stop=True,
                            )
                        nc.scalar.copy(o[0:cn, :, :], p[:])
                    else:
                        for gg in range(2):
                            p = ps.tile([cn, 4, pad], F32, tag="ps")
                            for g4 in range(4):
                                g = gg * 4 + g4
                                nc.tensor.matmul(
                                    p[:, g4, 0:fout],
                                    lhs_fm[0:kdim, g, off:off + cn], w,
                                    start=True, stop=True,
                                )
                            nc.scalar.copy(o[0:cn, gg * 4:gg * 4 + 4, :],
                                           p[:, :, 0:fout])
                    if c == 1 and extra_row is not None:
                        erb = extra_row[:, None, :].broadcast_to((1, GPD, fout))
                        nc.gpsimd.dma_start(out=o[C1:C1 + 1, :, :], in_=erb)
                    outs.append(o)
                return outs

            # ---- encoder (m1 was emitted early, overlapping the d chain) ---
            z1 = an_mult(m1, s_b1d[0:HIDDEN, :], "z1")
            m2 = w_mult(z1, s_W2, HIDDEN, HIDDEN, "m2")
            z2 = an_mult(m2, s_b2d[0:HIDDEN, :], "z2")

            # ---- fused [v | z2^T]: one matmul streaming [Wa | I64] ---------
            vz = w_mult(z2, s_WAID, HIDDEN, VZ, "vz",
                        extra_row=wpk[0:1, WP_BA:WP_BA + VZ], pad=128)
            s_v = [vz[0][:, :, 0:CLUSTERS], vz[1][:, :, 0:CLUSTERS]]
            z2n = [vz[0][:, :, CLUSTERS:VZ], vz[1][0:C1, :, CLUSTERS:VZ]]

            # ---- assignment: S = softmax(An @ v + ba), nm ------------------
            s_S = []
            for mc, mn in ((0, C0), (1, C1)):
                moff = 0 if mc == 0 else C0
                p = ps.tile([mn, GPD, CLUSTERS], F32, tag="ps")
                for g in range(GPD):
                    for c, cn, ck in ((0, C0, C0), (1, C1, C1 + 1)):
                        nc.tensor.matmul(
                            p[:, g, :], s_An[c][0:ck, g, moff:moff + mn],
                            s_v[c][0:ck, g, :], start=(c == 0), stop=(c == 1),
                        )
                e = act.tile([mn, GPD, CLUSTERS], F32, tag=f"e{mc}")
                nc.scalar.activation(e[:], p[:], AF.Exp)
                ssum = act.tile([mn, GPD], F32, tag=f"ssum{mc}")
                nc.vector.reduce_sum(out=ssum[:], in_=e[:], axis=mybir.AxisListType.X)
                rs = act.tile([mn, GPD], F32, tag=f"rs{mc}")
                nc.vector.reciprocal(rs[:], ssum[:])
                s = act.tile([mn, GPD, CLUSTERS], MMDT, tag=f"s{mc}")
                nc.vector.tensor_mul(s[:], e[:],
                                     rs[:][:, :, None].broadcast_to((mn, GPD, CLUSTERS)))
                s_S.append(s)

            # ---- AS = A @ S (raw adjacency), nm ----------------------------
            s_AS = []
            for mc, mn in ((0, C0), (1, C1)):
                moff = 0 if mc == 0 else C0
                p = ps.tile([mn, GPD, CLUSTERS], F32, tag="ps")
                for g in range(GPD):
                    for c, cn in ((0, C0), (1, C1)):
                        nc.tensor.matmul(
                            p[:, g, :], s_a[c][0:cn, g, moff:moff + mn],
                            s_S[c][0:cn, g, :], start=(c == 0), stop=(c == 1),
                        )
                o = act.tile([mn, GPD, CLUSTERS], MMDT, tag=f"as{mc}")
                nc.scalar.copy(o[:], p[:])
                s_AS.append(o)

            # ---- pooled col-degree row first: colsum(Ap) == colsum(AS)
            # (softmax rows sum to 1), so its rsqrt runs on DVE while the
            # Ap/Zp matmuls run on PE --------------------------------------
            p_cs = ps.tile([1, GPD * CLUSTERS], F32, tag="ps")
            nc.tensor.matmul(p_cs[:], s_onec[0:C0, :], s_AS[0][:],
                             start=True, stop=False)
            nc.tensor.matmul(p_cs[:], s_onec[0:C1, :], s_AS[1][:],
                             start=False, stop=True)
            lnu = act.tile([1, GPD * CLUSTERS], F32, tag="lnu")
            nc.scalar.activation(lnu[:], p_cs[:], AF.Ln, bias=1.0)
            dprow = act.tile([1, GPD * CLUSTERS], F32, tag="dprow")
            nc.scalar.activation(dprow[:], lnu[:], AF.Exp, scale=-0.5)

            # ---- Ap = S^T @ AS; Zp = z2n^T @ S -----------------------------
            p_ap = ps.tile([CLUSTERS, GPD, CLUSTERS], F32, tag="ps")
            for g in range(GPD):
                for c, cn in ((0, C0), (1, C1)):
                    nc.tensor.matmul(p_ap[:, g, :], s_S[c][0:cn, g, :],
                                     s_AS[c][0:cn, g, :], start=(c == 0), stop=(c == 1))

            p_zp = ps.tile([HIDDEN, GPD, CLUSTERS], F32, tag="ps")
            for g in range(GPD):
                for c, cn in ((0, C0), (1, C1)):
                    nc.tensor.matmul(p_zp[:, g, :], z2n[c][0:cn, g, :],
                                     s_S[c][0:cn, g, :], start=(c == 0), stop=(c == 1))
            s_Zp = act.tile([HIDDEN, GPD, CLUSTERS], MMDT, tag="zp")
            nc.vector.tensor_copy(s_Zp[:], p_zp[:])

            # ---- pooled GCN part 1: ZW = Zp @ Wp (before the dprow
            # broadcast so the PE is not blocked behind the DVE rsqrt) ------
            p_zw = ps.tile([CLUSTERS, GPD, HIDDEN], F32, tag="ps")
            for g in range(GPD):
                nc.tensor.matmul(p_zw[:, g, :], s_Zp[:, g, :], s_Wp,
                                 start=True, stop=True)
            s_ZW = act.tile([CLUSTERS, GPD, HIDDEN], MMDT, tag="zw")
            nc.vector.tensor_copy(s_ZW[:], p_zw[:])

            p_dpb = ps.tile([CLUSTERS, GPD * CLUSTERS], F32, tag="ps")
            nc.tensor.matmul(p_dpb[:], s_ones25, dprow[:], start=True, stop=True)
            s_dpT = p_dpb[:].rearrange("p (g j) -> p g j", g=GPD)

            # ---- pooled row-degree + Anp ----------------------------------
            degp = act.tile([CLUSTERS, GPD], F32, tag="degp")
            nc.vector.reduce_sum(out=degp[:], in_=p_ap[:], axis=mybir.AxisListType.X)
            lnp = act.tile([CLUSTERS, GPD], F32, tag="lnp")
            nc.scalar.activation(lnp[:], degp[:], AF.Ln, bias=1.0)
            dp = act.tile([CLUSTERS, GPD], F32, tag="dp")
            nc.scalar.activation(dp[:], lnp[:], AF.Exp, scale=-0.5)

            # Anp = dp_row * (Ap + I) * dp_col
            ahp = act.tile([CLUSTERS, GPD, CLUSTERS], F32, tag="ahp")
            id25b = s_id25[:, None, :].broadcast_to((CLUSTERS, GPD, CLUSTERS))
            nc.vector.tensor_add(ahp[:], p_ap[:], id25b)
            nc.vector.tensor_mul(ahp[:], ahp[:],
                                 dp[:][:, :, None].broadcast_to((CLUSTERS, GPD, CLUSTERS)))
            anp = act.tile([CLUSTERS, GPD, CLUSTERS], MMDT, tag="anp")
            nc.vector.tensor_mul(anp[:], ahp[:], s_dpT)

            # ---- pooled GCN part 2: H = relu(Anp @ ZW + bp), fm ------------
            p_h = ps.tile([HIDDEN, GPD, CLUSTERS], F32, tag="ps")
            for g in range(GPD):
                nc.tensor.matmul(p_h[:, g, :], s_ZW[:, g, :], anp[:, g, :],
                                 start=True, stop=True)
            s_H = act.tile([HIDDEN, GPD, CLUSTERS], F32, tag="h")
            nc.scalar.activation(s_H[:], p_h[:], AF.Relu, bias=s_bp)

            # ---- readout + classifier --------------------------------------
            s_G = act.tile([HIDDEN, GPD], F32, tag="g")
            nc.vector.reduce_sum(out=s_G[:], in_=s_H[:], axis=mybir.AxisListType.X)

            p_l = ps.tile([GPD, NUM_CLASSES], F32, tag="ps")
            nc.tensor.matmul(p_l[:], s_G[:], s_Wc, start=True, stop=True)
            s_out = act.tile([GPD, NUM_CLASSES], F32, tag="logits")
            nc.vector.tensor_add(s_out[:], p_l[:], s_bc)
            nc.sync.dma_start(out=out, in_=s_out[:])

    nc.compile()
    return nc


def make_in_maps(x, a, W1, b1, W2, b2, Wa, ba, Wp, bp, Wc, bc):
    import ml_dtypes
    npmm = np.dtype(ml_dtypes.bfloat16) if MMDT == BF16 else np.dtype(np.float32)

    x = np.ascontiguousarray(np.asarray(x, dtype=np.float32))
    a = np.asarray(a, dtype=np.float32)

    # diagonal 150x150 blocks of the batch adjacency, node free-dim padded
    # to NPGP=152 with zeros for DVE alignment
    ab = a.reshape(B_GRAPHS, NPG, B_GRAPHS, NPG)
    blocks = ab[np.arange(B_GRAPHS), :, np.arange(B_GRAPHS), :]  # [64, 150, 150]
    blocks_p = np.zeros((B_GRAPHS, NPG, NPGP), np.float32)
    blocks_p[:, :, 0:NPG] = blocks
    blocks = blocks_p.astype(npmm)



    wpk = np.zeros((N_FEAT, WP_COLS), npmm)
    wpk[:, WP_W1:WP_W1 + HIDDEN] = np.asarray(W1, np.float32).astype(npmm)
    W2m = np.asarray(W2, np.float32).astype(npmm)
    wpk[0:HIDDEN, WP_W2:WP_W2 + HIDDEN] = W2m
    wpk[HIDDEN:2 * HIDDEN, WP_W2:WP_W2 + HIDDEN] = W2m
    Wam = np.asarray(Wa, np.float32).astype(npmm)
    wpk[0:HIDDEN, WP_WAID:WP_WAID + CLUSTERS] = Wam
    wpk[HIDDEN:2 * HIDDEN, WP_WAID:WP_WAID + CLUSTERS] = Wam
    wpk[0:HIDDEN, WP_WAID + CLUSTERS:WP_WAID + VZ] = np.eye(HIDDEN, dtype=npmm)
    wpk[HIDDEN:2 * HIDDEN, WP_WAID + CLUSTERS:WP_WAID + VZ] = np.eye(HIDDEN, dtype=npmm)
    wpk[0:HIDDEN, WP_WP:WP_WP + HIDDEN] = np.asarray(Wp, np.float32).astype(npmm)
    wpk[0, WP_BA:WP_BA + CLUSTERS] = np.asarray(ba, np.float32).astype(npmm)
    wpk[:, WP_ID128:WP_ID128 + N_FEAT] = np.eye(N_FEAT, dtype=npmm)
    wpk[0, WP_ONES:WP_ONES + NPG] = 1.0
    wpk[:, WP_ONEC] = 1.0
    # identity chunks id0 [128,152] | id1 [22,152] at WP_IDP
    wpk[np.arange(C0), WP_IDP + np.arange(C0)] = 1.0
    wpk[np.arange(C1), WP_IDP + NPGP + C0 + np.arange(C1)] = 1.0

    fpk = np.zeros((N_FEAT, FP_COLS), np.float32)
    fpk[0:HIDDEN, FP_WC:FP_WC + NUM_CLASSES] = np.asarray(Wc, np.float32)
    fpk[0:HIDDEN, FP_B1] = np.asarray(b1, np.float32)
    fpk[HIDDEN:2 * HIDDEN, FP_B1] = np.asarray(b1, np.float32)
    fpk[0:HIDDEN, FP_B2] = np.asarray(b2, np.float32)
    fpk[HIDDEN:2 * HIDDEN, FP_B2] = np.asarray(b2, np.float32)
    fpk[0:HIDDEN, FP_BP] = np.asarray(bp, np.float32)
    fpk[0:GPD, FP_BC:FP_BC + NUM_CLASSES] = np.asarray(bc, np.float32)[None, :]
    fpk[0:CLUSTERS, FP_ID25:FP_ID25 + CLUSTERS] = np.eye(CLUSTERS, dtype=np.float32)
    fpk[0, FP_ONESR:FP_ONESR + CLUSTERS] = 1.0

    common = dict(wpk=wpk, fpk=fpk)

    in_maps = []
    for d in range(DEV):
        xd = x[d * GPD * NPG:(d + 1) * GPD * NPG]          # [1200, 128]
        xTd = xd.T.reshape(N_FEAT, GPD, NPG)
        xTp = np.zeros((N_FEAT, GPD, NPGP), np.float32)
        xTp[:, :, 0:NPG] = xTd
        bd = blocks[d * GPD:(d + 1) * GPD]                  # [8, 150, 152]
        bt = np.ascontiguousarray(bd.transpose(1, 0, 2))    # [150, 8, 152]
        in_maps.append(dict(
            xT=xTp.astype(npmm),
            a0=np.ascontiguousarray(bt[:C0]),
            a1=np.ascontiguousarray(bt[C0:]),
            **common,
        ))
    return in_maps


def kernel(x, a, seg_ids, num_graphs, W1, b1, W2, b2, Wa, ba, Wp, bp, Wc, bc,
           trace=False):
    if "nc" not in _CACHE:
        _CACHE["nc"] = build_nc()
    nc = _CACHE["nc"]
    in_maps = make_in_maps(x, a, W1, b1, W2, b2, Wa, ba, Wp, bp, Wc, bc)
    res = run_bass_kernel_spmd(nc, in_maps, core_ids=list(range(DEV)), trace=trace)
    logits = np.concatenate([r["out"] for r in res.results], axis=0)
    if trace:
        return logits, res
    return logits


# revision 24
# speedup vs baseline: 1.0241x; 1.0241x over previous
"""GCN + DiffPool kernel for Trainium2, data-parallel over graphs across 8 NeuronCores.

Model (per graph, n=150 nodes):
  Z1 = relu(An @ (x @ W1) + b1)          An = D^-1/2 (A+I) D^-1/2
  Z2 = relu(An @ (Z1 @ W2) + b2)
  S  = softmax(An @ (Z2 @ Wa) + ba)      [n, 25]
  Zp = S^T @ Z2 ; Ap = S^T @ (A @ S)
  H  = relu(Anp @ (Zp @ Wp) + bp)        pooled GCN, 25 cluster-nodes
  logits = (sum_rows H) @ Wc + bc

Sharding: 64 graphs -> 8 devices x 8 graphs. The batch adjacency is block
diagonal, so each device only receives its 8 graphs' 150x150 diagonal blocks
(packed into a [128,8,150] + [22,8,150] partition-chunk layout) and its node
rows of x (shipped feature-major). Everything is graph-local; the final [8,10]
logits per device are concatenated on host.

On-device layout convention:
  fm (feature-major): [feat_part, graph, node]  - used for W-multiplies (lhsT)
  nm (node-major):    [node_part, graph, feat]  - used for A-multiplies
A-multiplies contract over nodes, so node dim (150) is split into partition
chunks c0=[0:128], c1=[128:150]. The FULL symmetric normalization is folded
into An = (A+I) * d_i * d_j via a PE rank-1 outer product d d^T per graph;
activations stay unscaled so their PSUM->SBUF evacuations are plain copies
with no dependency on the degree chain.
"""

import numpy as np

import concourse.bass as bass
import concourse.mybir as mybir
import concourse.tile as tile
from concourse import bacc
from concourse.bass_utils import run_bass_kernel_spmd

F32 = mybir.dt.float32
BF16 = mybir.dt.bfloat16
AF = mybir.ActivationFunctionType
AL = mybir.AluOpType
U32 = mybir.dt.uint32

MMDT = BF16

N_NODES = 9600
N_FEAT = 128
HIDDEN = 64
CLUSTERS = 25
NUM_CLASSES = 10
B_GRAPHS = 64
NPG = 150            # nodes per graph
DEV = 8              # devices
GPD = 8              # graphs per device
C0, C1 = 128, 22     # node partition chunks (128 + 22 = 150)
NPGP = 152           # node free-dim padded to 4B alignment for DVE 2x mode
VZ = CLUSTERS + HIDDEN   # fused [v | z2^T] free width = 89

_CACHE = {}

# packed-constant column offsets (bf16 tensor wpk [128, WP_COLS])
WP_W1 = 0                     # [128, 64]
WP_W2 = WP_W1 + HIDDEN        # [64, 64]   at 64
WP_WAID = WP_W2 + HIDDEN      # [64, 25+64] = [Wa | I64] at 128
WP_WP = WP_WAID + VZ          # [64, 64]   at 217
WP_BA = WP_WP + HIDDEN        # [1, 25]    at 281
WP_ID128 = WP_BA + CLUSTERS   # [128, 128] at 306
WP_ONES = WP_ID128 + N_FEAT   # [1, 150]   at 434
WP_ONEC = WP_ONES + NPG       # [128, 1]   at 584
WP_IDP = WP_ONEC + 1          # [128, 2*152] identity chunks at 585
WP_COLS = WP_IDP + 2 * NPGP   # 889

# fp32 tensor fpk [128, FP_COLS]
FP_WC = 0                     # [64, 10]
FP_B1 = FP_WC + NUM_CLASSES   # [64, 1] at 10
FP_B2 = FP_B1 + 1
FP_BP = FP_B2 + 1
FP_BC = FP_BP + 1             # [8, 10] at 13
FP_ID25 = FP_BC + NUM_CLASSES # [25, 25] at 23
FP_ONESR = FP_ID25 + CLUSTERS # [1, 25] ones row at 48
FP_COLS = FP_ONESR + CLUSTERS # 73


def build_nc():
    nc = bacc.Bacc("TRN2", target_bir_lowering=False, debug=False, num_devices=DEV)

    def din(name, shape, dt=F32):
        return nc.dram_tensor(name, shape, dt, kind="ExternalInput").ap()

    xT = din("xT", [N_FEAT, GPD, NPGP], MMDT)
    a0 = din("a0", [C0, GPD, NPGP], MMDT)
    a1 = din("a1", [C1, GPD, NPGP], MMDT)
    wpk = din("wpk", [N_FEAT, WP_COLS], MMDT)
    fpk = din("fpk", [N_FEAT, FP_COLS], F32)
    out = nc.dram_tensor("out", [GPD, NUM_CLASSES], F32, kind="ExternalOutput").ap()

    with tile.TileContext(nc) as tc:
        with (
            tc.tile_pool(name="cst", bufs=1) as cst,
            tc.tile_pool(name="act", bufs=1) as act,
            tc.tile_pool(name="ps", bufs=6, space="PSUM") as ps,
            tc.tile_pool(name="psz", bufs=2, space="PSUM") as psz,
            tc.tile_pool(name="dram", bufs=1, space="DRAM") as dram,
        ):
            # ---- input DMAs: a0 first (heads the degree->An chain); no DMAs
            # on the scalar queue so the ACT table load overlaps the loads ---
            HG = GPD // 2
            s_a0 = cst.tile([C0, GPD, NPGP], MMDT, tag="a0")
            nc.sync.dma_start(out=s_a0[:, 0:HG, :], in_=a0[:, 0:HG, :])
            nc.scalar.dma_start(out=s_a0[:, HG:GPD, :], in_=a0[:, HG:GPD, :])
            s_xT = cst.tile([N_FEAT, GPD, NPGP], MMDT, tag="xT")
            nc.gpsimd.dma_start(out=s_xT[:], in_=xT)
            s_wpk = cst.tile([N_FEAT, WP_COLS], MMDT, tag="wpk")
            nc.gpsimd.dma_start(out=s_wpk[:], in_=wpk)
            s_a1 = cst.tile([C1, GPD, NPGP], MMDT, tag="a1")
            nc.sync.dma_start(out=s_a1[:], in_=a1)
            s_fpk = cst.tile([N_FEAT, FP_COLS], F32, tag="fpk")
            nc.sync.dma_start(out=s_fpk[:], in_=fpk)

            s_a = (s_a0, s_a1)
            s_idp = s_wpk[:, WP_IDP:WP_IDP + 2 * NPGP]
            s_id = (s_idp[:, 0:NPGP], s_idp[0:C1, NPGP:2 * NPGP])
            s_W1 = s_wpk[:, WP_W1:WP_W1 + HIDDEN]
            s_W2 = s_wpk[0:HIDDEN, WP_W2:WP_W2 + HIDDEN]
            s_W2o = s_wpk[HIDDEN:2 * HIDDEN, WP_W2:WP_W2 + HIDDEN]
            s_WAID = s_wpk[0:HIDDEN, WP_WAID:WP_WAID + VZ]
            s_WAIDo = s_wpk[HIDDEN:2 * HIDDEN, WP_WAID:WP_WAID + VZ]
            s_Wp = s_wpk[0:HIDDEN, WP_WP:WP_WP + HIDDEN]
            s_id128 = s_wpk[:, WP_ID128:WP_ID128 + N_FEAT]
            s_onec = s_wpk[:, WP_ONEC:WP_ONEC + 1]
            s_Wc = s_fpk[0:HIDDEN, FP_WC:FP_WC + NUM_CLASSES]
            s_b1d = s_fpk[0:2 * HIDDEN, FP_B1:FP_B1 + 1]
            s_b2d = s_fpk[0:2 * HIDDEN, FP_B2:FP_B2 + 1]
            s_bp = s_fpk[0:HIDDEN, FP_BP:FP_BP + 1]
            s_bc = s_fpk[0:GPD, FP_BC:FP_BC + NUM_CLASSES]
            s_id25 = s_fpk[0:CLUSTERS, FP_ID25:FP_ID25 + CLUSTERS]
            s_ones25 = s_fpk[0:1, FP_ONESR:FP_ONESR + CLUSTERS]

            # ---- degrees (DVE free-dim reduces; pads are zero) -------------
            degc = act.tile([C0, 2 * GPD], F32, tag="degc")
            nc.vector.memset(degc[0:C0, GPD:2 * GPD], 1.0)
            nc.vector.reduce_sum(out=degc[:, 0:HG], in_=s_a0[:, 0:HG, :],
                                 axis=mybir.AxisListType.X)
            nc.vector.reduce_sum(out=degc[:, HG:GPD], in_=s_a0[:, HG:GPD, :],
                                 axis=mybir.AxisListType.X)
            nc.vector.reduce_sum(out=degc[0:C1, GPD:2 * GPD], in_=s_a1[:],
                                 axis=mybir.AxisListType.X)
            qk1 = act.tile([C0, 1], U32, tag="qk1")
            nc.vector.memset(qk1[:], 1)
            qkm = act.tile([C0, 1], U32, tag="qkm")
            nc.vector.memset(qkm[:], 0x5F3759DF)

            # ---- m1 on PE while the rsqrt chain runs on DVE ----------------
            m1 = []
            for c, cn in ((0, C0), (1, C1)):
                off = 0 if c == 0 else C0
                o = act.tile([cn, GPD, HIDDEN], MMDT, tag=f"m1{c}")
                p = ps.tile([cn, GPD, HIDDEN], F32, tag="ps")
                for g in range(GPD):
                    nc.tensor.matmul(p[:, g, :], s_xT[0:N_FEAT, g, off:off + cn],
                                     s_W1, start=True, stop=True)
                nc.scalar.copy(o[:], p[:])
                m1.append(o)

            # ---- PE warm-up filler while the rsqrt chain runs on DVE -------
            p_wu = ps.tile([C0, 3 * NPGP], F32, tag="ps")
            for _ in range(6):
                nc.tensor.matmul(p_wu[:], s_id128, s_xT[:, 0:3, :],
                                 start=True, stop=True)

            # ---- d = rsqrt(deg+1): quake seed + 1 Newton step on DVE -------
            # (single iteration: ~0.2% max rel err, below bf16 quantization)
            def emit_rsqrt(x, rows, cols, out=None, odt=F32):
                s = act.tile([rows, cols], F32, tag=f"rs_{id(x)}")
                w = act.tile([rows, cols], F32, tag=f"rw_{id(x)}")
                nc.vector.tensor_tensor(s[:].bitcast(U32), x[:].bitcast(U32),
                                        qk1[0:rows, :].broadcast_to((rows, cols)),
                                        AL.logical_shift_right)
                nc.vector.tensor_tensor(s[:].bitcast(U32),
                                        qkm[0:rows, :].broadcast_to((rows, cols)),
                                        s[:].bitcast(U32), AL.subtract)
                nc.vector.tensor_mul(w[:], s[:], s[:])
                nc.vector.tensor_mul(w[:], w[:], x[:])
                nc.vector.tensor_scalar(w[:], w[:], -0.5, 1.5, AL.mult, AL.add)
                if out is None:
                    out = act.tile([rows, cols], odt, tag=f"ro_{id(x)}")
                nc.vector.tensor_mul(out[:], s[:], w[:])
                return out

            nc.vector.tensor_scalar_add(degc[:], degc[:], 1.0)
            dbfc = act.tile([C0, 2 * GPD], MMDT, tag="dbfc")
            emit_rsqrt(degc, C0, 2 * GPD, out=dbfc)

            # ---- dTrow[g, j] = d_g[j] via PE transposes, then a DRAM
            # roundtrip (split in graph halves, reads on two queues) ---------
            p_dt = ps.tile([GPD, C0], MMDT, tag="ps")
            nc.tensor.transpose(p_dt[:, 0:C0], dbfc[:, 0:GPD], s_id128)
            p_dt2 = ps.tile([GPD, C1], MMDT, tag="ps")
            nc.tensor.transpose(p_dt2[:, 0:C1], dbfc[0:C1, GPD:2 * GPD],
                                s_id128[0:C1, 0:C1])
            dTrow = act.tile([GPD, NPGP], MMDT, tag="dTrow")
            nc.vector.memset(dTrow[:, NPG:NPGP], 0)
            nc.vector.tensor_copy(dTrow[:, 0:C0], p_dt[:])
            nc.vector.tensor_copy(dTrow[:, C0:NPG], p_dt2[:])
            dTd = dram.tile([GPD * NPGP], MMDT, tag="dTd")
            dTd_gj = dTd[:].rearrange("(g j) -> g j", g=GPD)
            nc.sync.dma_start(out=dTd_gj[:], in_=dTrow[:])
            s_dT = cst.tile([C0, GPD, NPGP], MMDT, tag="dT")
            dT_src = dTd_gj[None, :, :].broadcast_to((C0, GPD, NPGP))
            nc.sync.dma_start(out=s_dT[:, 0:HG, :], in_=dT_src[:, 0:HG, :])
            nc.scalar.dma_start(out=s_dT[:, HG:GPD, :], in_=dT_src[:, HG:GPD, :])

            # ---- more warm-up while the dT roundtrip is in flight ----------
            for _ in range(8):
                nc.tensor.matmul(p_wu[:], s_id128, s_xT[:, 0:3, :],
                                 start=True, stop=True)

            # ---- An = (A+I) * d_col * d_row, bf16 (both scalings folded) ---
            ah0 = act.tile([C0, GPD, NPGP], MMDT, tag="ah0")
            idb0 = s_id[0][:, None, :].broadcast_to((C0, GPD, NPGP))
            nc.vector.tensor_add(ah0[:], s_a0[:], idb0)
            ah1 = act.tile([C1, GPD, NPGP], MMDT, tag="ah1")
            idb1 = s_id[1][:, None, :].broadcast_to((C1, GPD, NPGP))
            nc.vector.tensor_add(ah1[:], s_a1[:], idb1)

            # row scaling (broadcast operand, 1x mode) runs during the dT
            # roundtrip; the aligned 2x column scaling lands last.
            an0 = act.tile([C0, GPD, NPGP], MMDT, tag="an0")
            an1 = act.tile([C1 + 1, GPD, NPGP], MMDT, tag="an1")
            dbc0 = dbfc[:, 0:GPD][:, :, None].broadcast_to((C0, GPD, NPGP))
            dbc1 = dbfc[0:C1, GPD:2 * GPD][:, :, None].broadcast_to((C1, GPD, NPGP))
            nc.vector.tensor_mul(ah0[:], ah0[:], dbc0)
            nc.vector.tensor_mul(ah1[:], ah1[:], dbc1)
            for gg in range(2):
                gs = slice(gg * 4, gg * 4 + 4)
                nc.vector.tensor_mul(an0[:, gs, :], ah0[:, gs, :],
                                     s_dT[:, gs, :])
                nc.vector.tensor_mul(an1[0:C1, gs, :], ah1[:, gs, :],
                                     s_dT[0:C1, gs, :])
            s_An = [an0, an1]
            ones_src = wpk[0:1, WP_ONES:WP_ONES + NPG][:, None, :] \
                .broadcast_to((1, GPD, NPG))
            nc.gpsimd.dma_start(out=an1[C1:C1 + 1, :, 0:NPG], in_=ones_src)

            # ---- helpers ---------------------------------------------------
            def an_mult(m_nm, bias, name):
                """fm out [HIDDEN, g, NPG] = relu((An @ M) + bias), batched
                relu over 4-graph PSUM groups."""
                o = act.tile([HIDDEN, GPD, NPG], MMDT, tag=name)
                for gg in range(4):
                    p = psz.tile([HIDDEN, 2, 256], F32, tag="bigz")
                    for g2 in range(2):
                        g = gg * 2 + g2
                        for c, cn in ((0, C0), (1, C1)):
                            nc.tensor.matmul(
                                p[:, g2, 0:NPG], m_nm[c][0:cn, g, :],
                                s_An[c][0:cn, g, 0:NPG],
                                start=(c == 0), stop=(c == 1),
                            )
                    nc.scalar.activation(o[:, gg * 2:gg * 2 + 2, :],
                                         p[:, :, 0:NPG], AF.Relu, bias=bias)
                return o

            def w_mult(lhs_fm, w, kdim, fout, name, extra_row=None, pad=None):
                """nm out [node_chunk, g, fout] = (lhs_fm chunk)^T @ w, plain
                (no scaling).  PSUM->SBUF evacuation on the scalar engine."""
                outs = []
                for c, cn in ((0, C0), (1, C1)):
                    off = 0 if c == 0 else C0
                    rows = cn + (1 if (c == 1 and extra_row is not None) else 0)
                    o = act.tile([rows, GPD, fout], MMDT, tag=f"{name}{c}")
                    if pad is None:
                        p = ps.tile([cn, GPD, fout], F32, tag="ps")
                        for g in range(GPD):
                            nc.tensor.matmul(
                                p[:, g, :], lhs_fm[0:kdim, g, off:off + cn], w,
                                start=True, stop=True,
                            )
                        nc.scalar.copy(o[0:cn, :, :], p[:])
                    else:
                        for gg in range(2):
                            p = ps.tile([cn, 4, pad], F32, tag="ps")
                            for g4 in range(4):
                                g = gg * 4 + g4
                                nc.tensor.matmul(
                                    p[:, g4, 0:fout],
                                    lhs_fm[0:kdim, g, off:off + cn], w,
                                    start=True, stop=True,
                                )
                            nc.scalar.copy(o[0:cn, gg * 4:gg * 4 + 4, :],
                                           p[:, :, 0:fout])
                    if c == 1 and extra_row is not None:
                        erb = extra_row[:, None, :].broadcast_to((1, GPD, fout))
                        nc.gpsimd.dma_start(out=o[C1:C1 + 1, :, :], in_=erb)
                    outs.append(o)
                return outs

            # ---- encoder (m1 was emitted early, overlapping the d chain) ---
            z1 = an_mult(m1, s_b1d[0:HIDDEN, :], "z1")
            m2 = w_mult(z1, s_W2, HIDDEN, HIDDEN, "m2")
            z2 = an_mult(m2, s_b2d[0:HIDDEN, :], "z2")

            # ---- fused [v | z2^T]: one matmul streaming [Wa | I64] ---------
            vz = w_mult(z2, s_WAID, HIDDEN, VZ, "vz",
                        extra_row=wpk[0:1, WP_BA:WP_BA + VZ], pad=128)
            s_v = [vz[0][:, :, 0:CLUSTERS], vz[1][:, :, 0:CLUSTERS]]
            z2n = [vz[0][:, :, CLUSTERS:VZ], vz[1][0:C1, :, CLUSTERS:VZ]]

            # ---- assignment: S = softmax(An @ v + ba), nm ------------------
            s_S = []
            for mc, mn in ((0, C0), (1, C1)):
                moff = 0 if mc == 0 else C0
                p = ps.tile([mn, GPD, CLUSTERS], F32, tag="ps")
                for g in range(GPD):
                    for c, cn, ck in ((0, C0, C0), (1, C1, C1 + 1)):
                        nc.tensor.matmul(
                            p[:, g, :], s_An[c][0:ck, g, moff:moff + mn],
                            s_v[c][0:ck, g, :], start=(c == 0), stop=(c == 1),
                        )
                e = act.tile([mn, GPD, CLUSTERS], F32, tag=f"e{mc}")
                nc.scalar.activation(e[:], p[:], AF.Exp)
                ssum = act.tile([mn, GPD], F32, tag=f"ssum{mc}")
                nc.vector.reduce_sum(out=ssum[:], in_=e[:], axis=mybir.AxisListType.X)
                rs = act.tile([mn, GPD], F32, tag=f"rs{mc}")
                nc.vector.reciprocal(rs[:], ssum[:])
                s = act.tile([mn, GPD, CLUSTERS], MMDT, tag=f"s{mc}")
                nc.vector.tensor_mul(s[:], e[:],
                                     rs[:][:, :, None].broadcast_to((mn, GPD, CLUSTERS)))
                s_S.append(s)

            # ---- AS = A @ S (raw adjacency), nm ----------------------------
            s_AS = []
            for mc, mn in ((0, C0), (1, C1)):
                moff = 0 if mc == 0 else C0
                p = ps.tile([mn, GPD, CLUSTERS], F32, tag="ps")
                for g in range(GPD):
                    for c, cn in ((0, C0), (1, C1)):
                        nc.tensor.matmul(
                            p[:, g, :], s_a[c][0:cn, g, moff:moff + mn],
                            s_S[c][0:cn, g, :], start=(c == 0), stop=(c == 1),
                        )
                o = act.tile([mn, GPD, CLUSTERS], MMDT, tag=f"as{mc}")
                nc.scalar.copy(o[:], p[:])
                s_AS.append(o)

            # ---- pooled col-degree row first: colsum(Ap) == colsum(AS)
            # (softmax rows sum to 1), so its rsqrt runs on DVE while the
            # Ap/Zp matmuls run on PE --------------------------------------
            p_cs = ps.tile([1, GPD * CLUSTERS], F32, tag="ps")
            nc.tensor.matmul(p_cs[:], s_onec[0:C0, :], s_AS[0][:],
                             start=True, stop=False)
            nc.tensor.matmul(p_cs[:], s_onec[0:C1, :], s_AS[1][:],
                             start=False, stop=True)
            urow = act.tile([1, GPD * CLUSTERS], F32, tag="urow")
            nc.vector.tensor_scalar_add(urow[:], p_cs[:], 1.0)
            dprow = emit_rsqrt(urow, 1, GPD * CLUSTERS)

            # ---- Ap = S^T @ AS; Zp = z2n^T @ S -----------------------------
            p_ap = ps.tile([CLUSTERS, GPD, CLUSTERS], F32, tag="ps")
            for g in range(GPD):
                for c, cn in ((0, C0), (1, C1)):
                    nc.tensor.matmul(p_ap[:, g, :], s_S[c][0:cn, g, :],
                                     s_AS[c][0:cn, g, :], start=(c == 0), stop=(c == 1))

            p_zp = ps.tile([HIDDEN, GPD, CLUSTERS], F32, tag="ps")
            for g in range(GPD):
                for c, cn in ((0, C0), (1, C1)):
                    nc.tensor.matmul(p_zp[:, g, :], z2n[c][0:cn, g, :],
                                     s_S[c][0:cn, g, :], start=(c == 0), stop=(c == 1))
            s_Zp = act.tile([HIDDEN, GPD, CLUSTERS], MMDT, tag="zp")
            nc.vector.tensor_copy(s_Zp[:], p_zp[:])

            # ---- pooled GCN part 1: ZW = Zp @ Wp (before the dprow
            # broadcast so the PE is not blocked behind the DVE rsqrt) ------
            p_zw = ps.tile([CLUSTERS, GPD, HIDDEN], F32, tag="ps")
            for g in range(GPD):
                nc.tensor.matmul(p_zw[:, g, :], s_Zp[:, g, :], s_Wp,
                                 start=True, stop=True)
            s_ZW = act.tile([CLUSTERS, GPD, HIDDEN], MMDT, tag="zw")
            nc.vector.tensor_copy(s_ZW[:], p_zw[:])

            p_dpb = ps.tile([CLUSTERS, GPD * CLUSTERS], F32, tag="ps")
            nc.tensor.matmul(p_dpb[:], s_ones25, dprow[:], start=True, stop=True)
            s_dpT = p_dpb[:].rearrange("p (g j) -> p g j", g=GPD)

            # ---- pooled row-degree + Anp ----------------------------------
            degp = act.tile([CLUSTERS, GPD], F32, tag="degp")
            nc.vector.reduce_sum(out=degp[:], in_=p_ap[:], axis=mybir.AxisListType.X)
            nc.vector.tensor_scalar_add(degp[:], degp[:], 1.0)
            dp = emit_rsqrt(degp, CLUSTERS, GPD)

            # Anp = dp_row * (Ap + I) * dp_col
            ahp = act.tile([CLUSTERS, GPD, CLUSTERS], F32, tag="ahp")
            id25b = s_id25[:, None, :].broadcast_to((CLUSTERS, GPD, CLUSTERS))
            nc.vector.tensor_add(ahp[:], p_ap[:], id25b)
            nc.vector.tensor_mul(ahp[:], ahp[:],
                                 dp[:][:, :, None].broadcast_to((CLUSTERS, GPD, CLUSTERS)))
            anp = act.tile([CLUSTERS, GPD, CLUSTERS], MMDT, tag="anp")
            nc.vector.tensor_mul(anp[:], ahp[:], s_dpT)

            # ---- pooled GCN part 2: H = relu(Anp @ ZW + bp), fm ------------
            p_h = ps.tile([HIDDEN, GPD, CLUSTERS], F32, tag="ps")
            for g in range(GPD):
                nc.tensor.matmul(p_h[:, g, :], s_ZW[:, g, :], anp[:, g, :],
                                 start=True, stop=True)
            s_H = act.tile([HIDDEN, GPD, CLUSTERS], F32, tag="h")
            nc.scalar.activation(s_H[:], p_h[:], AF.Relu, bias=s_bp)

            # ---- readout + classifier --------------------------------------
            s_G = act.tile([HIDDEN, GPD], F32, tag="g")
            nc.vector.reduce_sum(out=s_G[:], in_=s_H[:], axis=mybir.AxisListType.X)

            p_l = ps.tile([GPD, NUM_CLASSES], F32, tag="ps")
            nc.tensor.matmul(p_l[:], s_G[:], s_Wc, start=True, stop=True)
            s_out = act.tile([GPD, NUM_CLASSES], F32, tag="logits")
            nc.vector.tensor_add(s_out[:], p_l[:], s_bc)
            nc.sync.dma_start(out=out, in_=s_out[:])

    nc.compile()
    return nc


def make_in_maps(x, a, W1, b1, W2, b2, Wa, ba, Wp, bp, Wc, bc):
    import ml_dtypes
    npmm = np.dtype(ml_dtypes.bfloat16) if MMDT == BF16 else np.dtype(np.float32)

    x = np.ascontiguousarray(np.asarray(x, dtype=np.float32))
    a = np.asarray(a, dtype=np.float32)

    # diagonal 150x150 blocks of the batch adjacency, node free-dim padded
    # to NPGP=152 with zeros for DVE alignment
    ab = a.reshape(B_GRAPHS, NPG, B_GRAPHS, NPG)
    blocks = ab[np.arange(B_GRAPHS), :, np.arange(B_GRAPHS), :]  # [64, 150, 150]
    blocks_p = np.zeros((B_GRAPHS, NPG, NPGP), np.float32)
    blocks_p[:, :, 0:NPG] = blocks
    blocks = blocks_p.astype(npmm)



    wpk = np.zeros((N_FEAT, WP_COLS), npmm)
    wpk[:, WP_W1:WP_W1 + HIDDEN] = np.asarray(W1, np.float32).astype(npmm)
    W2m = np.asarray(W2, np.float32).astype(npmm)
    wpk[0:HIDDEN, WP_W2:WP_W2 + HIDDEN] = W2m
    wpk[HIDDEN:2 * HIDDEN, WP_W2:WP_W2 + HIDDEN] = W2m
    Wam = np.asarray(Wa, np.float32).astype(npmm)
    wpk[0:HIDDEN, WP_WAID:WP_WAID + CLUSTERS] = Wam
    wpk[HIDDEN:2 * HIDDEN, WP_WAID:WP_WAID + CLUSTERS] = Wam
    wpk[0:HIDDEN, WP_WAID + CLUSTERS:WP_WAID + VZ] = np.eye(HIDDEN, dtype=npmm)
    wpk[HIDDEN:2 * HIDDEN, WP_WAID + CLUSTERS:WP_WAID + VZ] = np.eye(HIDDEN, dtype=npmm)
    wpk[0:HIDDEN, WP_WP:WP_WP + HIDDEN] = np.asarray(Wp, np.float32).astype(npmm)
    wpk[0, WP_BA:WP_BA + CLUSTERS] = np.asarray(ba, np.float32).astype(npmm)
    wpk[:, WP_ID128:WP_ID128 + N_FEAT] = np.eye(N_FEAT, dtype=npmm)
    wpk[0, WP_ONES:WP_ONES + NPG] = 1.0
    wpk[:, WP_ONEC] = 1.0
    # identity chunks id0 [128,152] | id1 [22,152] at WP_IDP
    wpk[np.arange(C0), WP_IDP + np.arange(C0)] = 1.0
    wpk[np.arange(C1), WP_IDP + NPGP + C0 + np.arange(C1)] = 1.0

    fpk = np.zeros((N_FEAT, FP_COLS), np.float32)
    fpk[0:HIDDEN, FP_WC:FP_WC + NUM_CLASSES] = np.asarray(Wc, np.float32)
    fpk[0:HIDDEN, FP_B1] = np.asarray(b1, np.float32)
    fpk[HIDDEN:2 * HIDDEN, FP_B1] = np.asarray(b1, np.float32)
    fpk[0:HIDDEN, FP_B2] = np.asarray(b2, np.float32)
    fpk[HIDDEN:2 * HIDDEN, FP_B2] = np.asarray(b2, np.float32)
    fpk[0:HIDDEN, FP_BP] = np.asarray(bp, np.float32)
    fpk[0:GPD, FP_BC:FP_BC + NUM_CLASSES] = np.asarray(bc, np.float32)[None, :]
    fpk[0:CLUSTERS, FP_ID25:FP_ID25 + CLUSTERS] = np.eye(CLUSTERS, dtype=np.float32)
    fpk[0, FP_ONESR:FP_ONESR + CLUSTERS] = 1.0

    common = dict(wpk=wpk, fpk=fpk)

    in_maps = []
    for d in range(DEV):
        xd = x[d * GPD * NPG:(d + 1) * GPD * NPG]          # [1200, 128]
        xTd = xd.T.reshape(N_FEAT, GPD, NPG)
        xTp = np.zeros((N_FEAT, GPD, NPGP), np.float32)
        xTp[:, :, 0:NPG] = xTd
        bd = blocks[d * GPD:(d + 1) * GPD]                  # [8, 150, 152]
        bt = np.ascontiguousarray(bd.transpose(1, 0, 2))    # [150, 8, 152]
        in_maps.append(dict(
            xT=xTp.astype(npmm),
            a0=np.ascontiguousarray(bt[:C0]),
            a1=np.ascontiguousarray(bt[C0:]),
            **common,
        ))
    return in_maps


def kernel(x, a, seg_ids, num_graphs, W1, b1, W2, b2, Wa, ba, Wp, bp, Wc, bc,
           trace=False):
    if "nc" not in _CACHE:
        _CACHE["nc"] = build_nc()
    nc = _CACHE["nc"]
    in_maps = make_in_maps(x, a, W1, b1, W2, b2, Wa, ba, Wp, bp, Wc, bc)
    res = run_bass_kernel_spmd(nc, in_maps, core_ids=list(range(DEV)), trace=trace)
    logits = np.concatenate([r["out"] for r in res.results], axis=0)
    if trace:
        return logits, res
    return logits
